# revision 1
# baseline (speedup 1.0000x reference)
"""Self-contained Trainium2 kernel for nn_EquiformerV2_46420006535674.

kernel(**inputs) -> np.ndarray [5000, 49, 32]

Strategy: 8-way SPMD over the chip's NeuronCores. Nodes sharded 625/core
(5 blocks x 128). Edges whose fp32 Gaussian-smearing row is identically
zero (beyond radial cutoff support) contribute exactly +1 to their dst's
softmax denominator and nothing else; they are counted host-side. The
~12k remaining active edges are sharded by dst block, padded to 384
slots/block. Per layer: f-major RMS-norm stats via masked matmuls,
AllGather of normalized features, indirect-DMA edge gathers, accumulating
PE transposes (msg = x1[src]+x1[dst] summed in PSUM), radial MLP from an
on-the-fly Gaussian basis, blockdiagonal per-degree einsums, single-pass
softmax (exp(logit) scattered alongside weighted values via one-hot
matmuls; denominator divided post-aggregation), and a gated FFN.
fp32 PE transposes (exact), float32r heavy matmuls, PSUM-bank-aligned
matmul output chunks.
"""
import numpy as np
from contextlib import ExitStack

import concourse.bass as bass
import concourse.mybir as mybir
import concourse.tile as tile
from concourse.masks import make_identity
from concourse.bass_utils import run_bass_kernel_spmd


# ================= host preprocessing =================


N, E, C, K, NL = 5000, 40000, 32, 49, 7
H, A, G, F, ECH = 8, 32, 600, 128, 128
ZMAX, CUTOFF = 90, 5.0
LYR = 2
NCORE = 8
NNODE = N // NCORE          # 625
NPAD = 640                  # 5 * 128
NBLK = 5
KPAD = 52
NCH = 13                    # feature chunks (4 k x 32 c)
FPAD = KPAD * C             # 1664
ET = 3
EC = ET * 128               # 384
E_PAD = NBLK * EC           # 1920
NTILES = E_PAD // 128       # 15

L_IDX = np.repeat(np.arange(NL), 2 * np.arange(NL) + 1)
L_IDX_PAD = np.concatenate([L_IDX, np.full(KPAD - K, NL - 1)])

_offs = np.linspace(0.0, CUTOFF, G).astype(np.float32)
_coeff = np.float32(-0.5 / (2.0 * (CUTOFF / (G - 1))) ** 2)


def preprocess(inputs):
    src = np.asarray(inputs["edge_index"][0]).astype(np.int64)
    dst = np.asarray(inputs["edge_index"][1]).astype(np.int64)
    pos = np.asarray(inputs["pos"]).astype(np.float32)
    vec = pos[dst] - pos[src]
    dist = np.sqrt((vec * vec).sum(-1) + np.float32(1e-12)).astype(np.float32)

    # active iff the fp32 smear row is not exactly zero
    act = (dist <= CUTOFF) | (np.exp(_coeff * (dist - _offs[-1]) ** 2,
                                     dtype=np.float32) > 0)

    core_of = dst // NNODE
    blk_of = (dst % NNODE) // 128
    loc_of = (dst % NNODE) % 128

    srcg = np.zeros((NCORE, NTILES, 128), np.int32)
    dstg = np.zeros((NCORE, NTILES, 128), np.int32)
    dist_pad = np.full((NCORE, E_PAD), 100.0, np.float32)
    S = np.zeros((NCORE, NTILES, 128, 128), np.float32)
    icnt = np.full((NCORE, 128, NBLK), 1e-9, np.float32)

    def g(n):
        return (n // NNODE) * NPAD + (n % NNODE)

    for c in range(NCORE):
        for b in range(NBLK):
            m = act & (core_of == c) & (blk_of == b)
            idx = np.nonzero(m)[0]
            idx = idx[np.argsort(loc_of[idx], kind="stable")]
            cnt = len(idx)
            assert cnt <= EC, f"core {c} block {b}: {cnt} > {EC}"
            base = b * EC
            flat_s = srcg[c].reshape(-1)
            flat_d = dstg[c].reshape(-1)
            flat_s[base:base + cnt] = g(src[idx])
            flat_d[base:base + cnt] = g(dst[idx])
            dist_pad[c, base:base + cnt] = dist[idx]
            Sf = S[c].reshape(E_PAD, 128)
            Sf[base + np.arange(cnt), loc_of[idx]] = 1.0
        mi = (~act) & (core_of == c)
        cnts = np.bincount(dst[mi] % NNODE, minlength=NPAD).astype(np.float32)
        icnt[c] += cnts.reshape(NBLK, 128).T

    distb = np.broadcast_to(
        dist_pad.reshape(NCORE, NBLK, 1, EC), (NCORE, NBLK, 120, EC)).copy()

    emb = np.asarray(inputs["emb_table"]).astype(np.float32)
    an = np.asarray(inputs["atomic_numbers"]).astype(np.int64)
    x0 = emb[an]
    x0T = np.zeros((NCORE, C, NPAD), np.float32)
    for c in range(NCORE):
        x0T[c, :, :NNODE] = x0[c * NNODE:(c + 1) * NNODE].T

    return dict(srcg=srcg, dstg=dstg, distb=distb, S=S, icnt=icnt, x0T=x0T)


def prep_weights(inputs):
    """Fold norm gains into consumers; emit partition-major weight layouts."""
    w = {k: np.asarray(v).astype(np.float32) for k, v in inputs.items()
         if k not in ("atomic_numbers", "pos", "edge_index")}
    out = {}
    for i in range(LYR):
        g1 = w["norm1_g"][i]
        g2 = w["norm2_g"][i]
        out[f"w1g_{i}"] = np.transpose(
            w["rad_w1"][i].reshape(5, 120, ECH), (1, 0, 2)).copy()   # [120,5,ECH]
        out[f"w2_{i}"] = w["rad_w2"][i].copy()                       # [ECH,ECH]
        w3 = w["rad_w3"][i]
        w3x = np.zeros((ECH, NCH, 128), np.float32)
        for m in range(NCH):
            for j in range(4):
                k = 4 * m + j
                if k < K:
                    l = L_IDX[k]
                    w3x[:, m, j * C:(j + 1) * C] = w3[:, l * C:(l + 1) * C]
        out[f"w3x_{i}"] = w3x
        out[f"wa1_{i}"] = (g1[0][:, None] * w["wa1"][i]).copy()      # [C, H*A]
        wa2 = w["wa2"][i]
        flat = np.zeros((H * A, H), np.float32)
        for h in range(H):
            flat[h * A:(h + 1) * A, h] = wa2[h]
        out[f"wa2b_{i}"] = np.stack([flat[:128], flat[128:]], 1)     # [128,2,H]
        wmsgb = np.zeros((128, NCH, 128), np.float32)
        woutb = np.zeros((128, NCH, 128), np.float32)
        for m in range(NCH):
            for j in range(4):
                k = 4 * m + j
                if k >= K:
                    continue
                l = L_IDX[k]
                wmsgb[j * C:(j + 1) * C, m, j * C:(j + 1) * C] = \
                    g1[l][:, None] * w["w_msg"][i][l]
                woutb[j * C:(j + 1) * C, m, j * C:(j + 1) * C] = w["w_out"][i][l]
        out[f"wmsgb_{i}"] = wmsgb
        out[f"woutb_{i}"] = woutb
        wf1 = np.zeros((128, KPAD, F), np.float32)
        wf2 = np.zeros((F, KPAD, 128), np.float32)
        for k in range(K):
            l = L_IDX[k]
            m, j = k // 4, k % 4
            wf1[j * C:(j + 1) * C, k, :] = g2[l][:, None] * w["wf1"][i][l]
            wf2[:, k, j * C:(j + 1) * C] = w["wf2"][i][l]
        out[f"wf1_{i}"] = wf1
        out[f"wf2_{i}"] = wf2
        out[f"wg_{i}"] = (g2[0][:, None] * w["wg"][i]).copy()        # [C, F]

    statm = np.zeros((128, NCH, NL), np.float32)
    expm = np.zeros((NL, NCH, 128), np.float32)
    for m in range(NCH):
        for j in range(4):
            k = 4 * m + j
            if k >= K:
                continue
            l = L_IDX[k]
            statm[j * C:(j + 1) * C, m, l] = 1.0 / ((2 * l + 1) * C)
            expm[l, m, j * C:(j + 1) * C] = 1.0
    out["statm"] = statm
    out["expm"] = expm
    out["offs_neg"] = (-_offs.reshape(5, 120).T).copy()              # [120, 5]
    return out


def make_in_maps(inputs):
    pp = preprocess(inputs)
    ww = prep_weights(inputs)
    in_maps = []
    for c in range(NCORE):
        m = dict(x0T=pp["x0T"][c], srcg=pp["srcg"][c], dstg=pp["dstg"][c],
                 distb=pp["distb"][c], S=pp["S"][c], icnt=pp["icnt"][c])
        m.update(ww)
        in_maps.append(m)
    return in_maps


def unshard(results):
    """results: list of 8 dicts with xout [NCH, 128, NPAD] -> [N, K, C]."""
    out = np.zeros((N, K, C), np.float32)
    for c in range(NCORE):
        arr = results[c]["xout"]                      # [NCH, 128, NPAD]
        xc = np.concatenate([arr[m].T for m in range(NCH)], axis=1)  # [NPAD, FPAD]
        out[c * NNODE:(c + 1) * NNODE] = xc[:NNODE, :K * C].reshape(NNODE, K, C)
    return out


# ================= multi-wait legalization =================

def split_multiwaits(nc):
    """This walrus build's codegen only supports 1 sync wait per instruction.
    For any instruction with >1 waits, hoist all but the last onto inserted
    single-wait EventSemaphore instructions (same engine, immediately before:
    engine programs execute serially, so the waits still complete before the
    original instruction issues)."""
    for fn in nc.m.functions:
        for b in fn.blocks:
            insts = b.instructions
            newlist = []
            changed = False
            for ins in insts:
                si = ins.sync_info
                if si is not None and len(si.on_wait) > 1:
                    waits = list(si.on_wait)
                    for k, w in enumerate(waits[:-1]):
                        ev = mybir.InstEventSemaphore(name=f"{ins.name}_w{k}")
                        ev.engine = ins.engine
                        ev.sync_info = mybir.SyncInfo(on_wait=[w], on_update=[])
                        newlist.append(ev)
                    ins.sync_info = mybir.SyncInfo(on_wait=[waits[-1]],
                                                   on_update=list(si.on_update))
                    changed = True
                newlist.append(ins)
            if changed:
                insts.clear()
                insts.extend(newlist)


# ================= device kernel builder =================

F32 = mybir.dt.float32
F32R = mybir.dt.float32r
I32 = mybir.dt.int32
AF = mybir.ActivationFunctionType
OP = mybir.AluOpType

COEFF = float(-0.5 / (2.0 * (CUTOFF / (G - 1))) ** 2)


def chunk_slices(total, bank=512):
    """PSUM-bank-aligned output chunks: a matmul's output must lie within a
    single 512-fp32 PSUM bank (walrus silently corrupts otherwise)."""
    out = []
    o = 0
    while o < total:
        w = min(bank, total - o)
        out.append((o, w))
        o += w
    return out


def build_nc(repeat=1):
    nc = bass.Bass(num_devices=NCORE)

    din = {}
    def inp(name, shape, dtype=F32):
        din[name] = nc.dram_tensor(name, list(shape), dtype, kind="ExternalInput")
        return din[name]

    inp("x0T", [C, NPAD])
    inp("srcg", [NTILES, 128], I32)
    inp("dstg", [NTILES, 128], I32)
    inp("distb", [NBLK, 120, EC])
    inp("S", [NTILES, 128, 128])
    inp("icnt", [128, NBLK])
    inp("offs_neg", [120, 5])
    inp("statm", [128, NCH, NL])
    inp("expm", [NL, NCH, 128])
    for i in range(LYR):
        inp(f"w1g_{i}", [120, 5, ECH])
        inp(f"w2_{i}", [ECH, ECH])
        inp(f"w3x_{i}", [ECH, NCH, 128])
        inp(f"wa1_{i}", [C, H * A])
        inp(f"wa2b_{i}", [128, 2, H])
        inp(f"wmsgb_{i}", [128, NCH, 128])
        inp(f"woutb_{i}", [128, NCH, 128])
        inp(f"wf1_{i}", [128, KPAD, F])
        inp(f"wf2_{i}", [F, KPAD, 128])
        inp(f"wg_{i}", [C, F])

    xout = nc.dram_tensor("xout", [NCH, 128, NPAD], F32, kind="ExternalOutput")
    dbg = {}
    for nm, shp in [("dbg_x1nm", [128, FPAD]), ("dbg_gs", [128, FPAD]),
                    ("dbg_r2", [ECH, EC]), ("dbg_msgr", [128, EC]),
                    ("dbg_ex", [H, EC]), ("dbg_vem", [128, FPAD + H]),
                    ("dbg_aggnm", [128, FPAD]), ("dbg_rden", [128, H]),
                    ("dbg_xl0", [NCH, 128, NPAD])]:
        dbg[nm] = nc.dram_tensor(nm, shp, F32, kind="ExternalOutput")

    with tile.TileContext(nc) as tc, ExitStack() as ctx:
        ctx.enter_context(nc.allow_low_precision(
            reason="float32r outputs are intentional rounding for fp32r matmuls"))
        const = ctx.enter_context(tc.tile_pool(name="const", bufs=1))
        xpool = ctx.enter_context(tc.tile_pool(name="x", bufs=1))
        dram = ctx.enter_context(tc.tile_pool(name="dram", bufs=1, space="DRAM"))

        def cload(name, shape, dtype=F32):
            t = const.tile(list(shape), dtype, tag=name)
            nc.gpsimd.dma_start(t[:], din[name][:])
            return t

        statm = cload("statm", [128, NCH, NL], F32R)
        expm = cload("expm", [NL, NCH, 128], F32R)
        icnt_t = cload("icnt", [128, NBLK], F32)
        offs_t = cload("offs_neg", [120, 5], F32)
        ident = const.tile([128, 128], F32, tag="ident")
        make_identity(nc, ident[:])
        c1e5 = const.tile([128, 1], F32, tag="c1e5")
        nc.gpsimd.memset(c1e5[:], 1e-5)

        x = [xpool.tile([128, NPAD], F32, tag=f"x{m}", name=f"x{m}") for m in range(NCH)]
        for m in range(NCH):
            nc.gpsimd.memset(x[m][:], 0.0)
        nc.sync.dma_start(x[0][0:C, :], din["x0T"][:])

        cc_outs = [dram.tile([NCORE * NPAD, FPAD], F32, tag=f"cc_out{i}",
                             name=f"cc_out{i}", addr_space="Shared")
                   for i in range(LYR * repeat)]
        cc_ins = [dram.tile([NPAD, FPAD], F32, tag=f"cc_in{i}", name=f"cc_in{i}")
                  for i in range(LYR * repeat)]

        for rep_i in range(LYR * repeat):
            i = rep_i % LYR
            cc_in = cc_ins[rep_i]
            cc_out = cc_outs[rep_i]
            with tc.tile_pool(name=f"we{i}", bufs=1) as wepool:
                wr = {}
                for nm, shp in [(f"w1g_{i}", [120, 5, ECH]), (f"w2_{i}", [ECH, ECH]),
                                (f"w3x_{i}", [ECH, NCH, 128]), (f"wa1_{i}", [C, H * A]),
                                (f"wa2b_{i}", [128, 2, H]), (f"wmsgb_{i}", [128, NCH, 128]),
                                (f"woutb_{i}", [128, NCH, 128])]:
                    t = wepool.tile(shp, F32R, tag=nm, name=nm)
                    nc.gpsimd.dma_start(t[:], din[nm][:])
                    wr[nm.rsplit("_", 1)[0]] = t

                # ================= norm1 =================
                with tc.tile_pool(name="n1s", bufs=2) as sp, \
                     tc.tile_pool(name="n1p", bufs=2, space="PSUM") as pp, \
                     tc.tile_pool(name="n1st", bufs=1, space="PSUM") as pstat:
                    stat_ps = pstat.tile([NL, NPAD], F32, tag="stat")
                    for m in range(NCH):
                        xsq = sp.tile([128, NPAD], F32R, tag="xsq")
                        nc.scalar.activation(xsq[:], x[m][:], AF.Square)
                        for (o, w) in chunk_slices(NPAD):
                            nc.tensor.matmul(stat_ps[:, o:o + w], statm[:, m, :],
                                             xsq[:, o:o + w],
                                             start=(m == 0), stop=(m == NCH - 1))
                    sstat = sp.tile([NL, NPAD], F32, tag="sstat")
                    nc.scalar.activation(sstat[:], stat_ps[:], AF.Sqrt, bias=c1e5[0:NL, :])
                    inv1 = sp.tile([NL, NPAD], F32, tag="inv1")
                    nc.vector.reciprocal(inv1[:], sstat[:])

                    for b in range(NBLK):
                        ip = pp.tile([128, NL], F32, tag="invT")
                        nc.tensor.transpose(ip[:], inv1[:, b * 128:(b + 1) * 128],
                                            ident[0:NL, 0:NL])
                        inv_nm = sp.tile([128, NL], F32, tag="invnm")
                        nc.vector.tensor_copy(inv_nm[:], ip[:])
                        x1nm = sp.tile([128, FPAD], F32, tag="x1nm")
                        for m in range(NCH):
                            p = pp.tile([128, 128], F32, tag="xT")
                            nc.tensor.transpose(p[:], x[m][:, b * 128:(b + 1) * 128],
                                                ident[:])
                            nc.vector.tensor_copy(
                                x1nm[:, m * 128:(m + 1) * 128], p[:])
                        for l in range(NL):
                            lo, hi = l * l * C, (l + 1) * (l + 1) * C
                            nc.vector.tensor_scalar_mul(
                                x1nm[:, lo:hi], x1nm[:, lo:hi], inv_nm[:, l:l + 1])
                        nc.sync.dma_start(cc_in[b * 128:(b + 1) * 128, :], x1nm[:])
                        if i == 0 and b == 0:
                            nc.sync.dma_start(dbg["dbg_x1nm"][:, :], x1nm[:])

                # ================= AllGather =================
                nc.gpsimd.collective_compute(
                    "AllGather", OP.bypass,
                    replica_groups=[list(range(NCORE))],
                    ins=[cc_in.opt()], outs=[cc_out.opt()],
                )

                # ================= edge pass =================
                with tc.tile_pool(name="eg", bufs=3) as egp, \
                     tc.tile_pool(name="ev", bufs=1) as evp, \
                     tc.tile_pool(name="em", bufs=2) as emp, \
                     tc.tile_pool(name="es", bufs=4) as esp, \
                     tc.tile_pool(name="eagg", bufs=2) as eaggp, \
                     tc.tile_pool(name="pse", bufs=4, space="PSUM") as pse, \
                     tc.tile_pool(name="psagg", bufs=1, space="PSUM") as psagg:
                    for b in range(NBLK):
                        gsrc, gdst = [], []
                        for t in range(ET):
                            ti = b * ET + t
                            isrc = esp.tile([128, 1], I32, tag="isrc")
                            nc.sync.dma_start(isrc[:], din["srcg"][ti, :, None])
                            idst = esp.tile([128, 1], I32, tag="idst")
                            nc.sync.dma_start(idst[:], din["dstg"][ti, :, None])
                            gs = egp.tile([128, FPAD], F32, tag="gsrc")
                            nc.gpsimd.indirect_dma_start(
                                out=gs[:], out_offset=None, in_=cc_out[:],
                                in_offset=bass.IndirectOffsetOnAxis(
                                    ap=isrc[:, :1], axis=0))
                            gd = egp.tile([128, FPAD], F32, tag="gdst")
                            nc.gpsimd.indirect_dma_start(
                                out=gd[:], out_offset=None, in_=cc_out[:],
                                in_offset=bass.IndirectOffsetOnAxis(
                                    ap=idst[:, :1], axis=0))
                            gsrc.append(gs)
                            gdst.append(gd)
                            if i == 0 and b == 0 and t == 0:
                                nc.sync.dma_start(dbg["dbg_gs"][:, :], gs[:])

                        Sb = []
                        for t in range(ET):
                            st = esp.tile([128, 128], F32R, tag=f"S{t}")
                            nc.gpsimd.dma_start(st[:], din["S"][b * ET + t])
                            Sb.append(st)
                        distb = emp.tile([120, EC], F32, tag="distb", bufs=1)
                        nc.sync.dma_start(distb[:], din["distb"][b])

                        # radial MLP (f-major)
                        smearT = [emp.tile([120, EC], F32R, tag=f"smear{gc}", name=f"smear{gc}", bufs=1)
                                  for gc in range(5)]
                        for gc in range(5):
                            sh = emp.tile([120, EC], F32, tag="smsh")
                            nc.vector.tensor_scalar_add(sh[:], distb[:],
                                                        offs_t[:, gc:gc + 1])
                            sq = emp.tile([120, EC], F32, tag="smsq")
                            nc.vector.tensor_tensor(out=sq[:], in0=sh[:],
                                                    in1=sh[:], op=OP.mult)
                            nc.scalar.activation(smearT[gc][:], sq[:], AF.Exp,
                                                 scale=COEFF)
                        r1_ps = pse.tile([ECH, EC], F32, tag="pse")
                        for gc in range(5):
                            for (o, w) in chunk_slices(EC):
                                nc.tensor.matmul(r1_ps[:, o:o + w],
                                                 wr["w1g"][:, gc, :],
                                                 smearT[gc][:, o:o + w],
                                                 start=(gc == 0), stop=(gc == 4))
                        r1 = emp.tile([ECH, EC], F32R, tag="r1")
                        nc.scalar.activation(r1[:], r1_ps[:], AF.Silu)
                        r2_ps = pse.tile([ECH, EC], F32, tag="pse")
                        for (o, w) in chunk_slices(EC):
                            nc.tensor.matmul(r2_ps[:, o:o + w], wr["w2"],
                                             r1[:, o:o + w], start=True, stop=True)
                        r2 = emp.tile([ECH, EC], F32R, tag="r2")
                        nc.scalar.activation(r2[:], r2_ps[:], AF.Silu)
                        if i == 0 and b == 0:
                            nc.sync.dma_start(dbg["dbg_r2"][:, :], r2[:].bitcast(F32))

                        val_em = [evp.tile([128, FPAD + H], F32R, tag=f"vem{t}", name=f"vem{t}")
                                  for t in range(ET)]
                        ex_em = None
                        for m in range(NCH):
                            msgT_ps = pse.tile([128, EC], F32, tag="pse")
                            for t in range(ET):
                                nc.tensor.matmul(
                                    msgT_ps[:, t * 128:(t + 1) * 128],
                                    gsrc[t][:, m * 128:(m + 1) * 128], ident[:],
                                    is_transpose=True, start=True, stop=False)
                                nc.tensor.matmul(
                                    msgT_ps[:, t * 128:(t + 1) * 128],
                                    gdst[t][:, m * 128:(m + 1) * 128], ident[:],
                                    is_transpose=True, start=False, stop=True)
                            msgT = emp.tile([128, EC], F32, tag="msgT")
                            nc.vector.tensor_copy(msgT[:], msgT_ps[:])
                            rexp_ps = pse.tile([128, EC], F32, tag="pse")
                            for (o, w) in chunk_slices(EC):
                                nc.tensor.matmul(rexp_ps[:, o:o + w],
                                                 wr["w3x"][:, m, :],
                                                 r2[:, o:o + w], start=True, stop=True)
                            msgr = emp.tile([128, EC], F32R, tag="msgr")
                            nc.vector.tensor_tensor(out=msgr[:], in0=msgT[:],
                                                    in1=rexp_ps[:], op=OP.mult)
                            valT_ps = pse.tile([128, EC], F32, tag="pse")
                            for (o, w) in chunk_slices(EC):
                                nc.tensor.matmul(valT_ps[:, o:o + w],
                                                 wr["wmsgb"][:, m, :],
                                                 msgr[:, o:o + w],
                                                 start=True, stop=True)
                            valTs = emp.tile([128, EC], F32, tag="valTs")
                            nc.vector.tensor_copy(valTs[:], valT_ps[:])
                            for t in range(ET):
                                vp = pse.tile([128, 128], F32, tag="pse")
                                nc.tensor.transpose(vp[:],
                                                    valTs[:, t * 128:(t + 1) * 128],
                                                    ident[:])
                                nc.vector.tensor_copy(
                                    val_em[t][:, m * 128:(m + 1) * 128], vp[:])

                            if m == 0 and i == 0 and b == 0:
                                nc.sync.dma_start(dbg["dbg_msgr"][:, :],
                                                  msgr[:].bitcast(F32))
                            if m == 0:
                                aTs = []
                                for half in range(2):
                                    aT_ps = pse.tile([128, EC], F32, tag="pse")
                                    for (o, w) in chunk_slices(EC):
                                        nc.tensor.matmul(
                                            aT_ps[:, o:o + w],
                                            wr["wa1"][:, half * 128:(half + 1) * 128],
                                            msgr[0:C, o:o + w],
                                            start=True, stop=True)
                                    aa = emp.tile([128, EC], F32R, tag=f"aT{half}")
                                    nc.scalar.activation(aa[:], aT_ps[:], AF.Silu)
                                    aTs.append(aa)
                                log_ps = pse.tile([H, EC], F32, tag="pse")
                                for half in range(2):
                                    for (o, w) in chunk_slices(EC):
                                        nc.tensor.matmul(log_ps[:, o:o + w],
                                                         wr["wa2b"][:, half, :],
                                                         aTs[half][:, o:o + w],
                                                         start=(half == 0),
                                                         stop=(half == 1))
                                exT = esp.tile([H, EC], F32, tag="exT")
                                nc.scalar.activation(exT[:], log_ps[:], AF.Exp)
                                if i == 0 and b == 0:
                                    nc.sync.dma_start(dbg["dbg_ex"][:, :], exT[:])
                                ex_em = []
                                for t in range(ET):
                                    ep = pse.tile([128, H], F32, tag="pse")
                                    nc.tensor.transpose(
                                        ep[:], exT[:, t * 128:(t + 1) * 128],
                                        ident[0:H, 0:H])
                                    ee = esp.tile([128, H], F32, tag=f"exem{t}")
                                    nc.vector.tensor_copy(ee[:], ep[:])
                                    ex_em.append(ee)

                        for t in range(ET):
                            vap = val_em[t][:, 0:FPAD].rearrange(
                                "p (k hh c) -> p k hh c", k=KPAD, hh=H, c=4)
                            nc.vector.tensor_tensor(
                                out=vap, in0=vap,
                                in1=ex_em[t][:, None, :, None].to_broadcast(
                                    [128, KPAD, H, 4]),
                                op=OP.mult)
                            nc.vector.tensor_copy(val_em[t][:, FPAD:FPAD + H],
                                                  ex_em[t][:])
                            if i == 0 and b == 0 and t == 0:
                                nc.sync.dma_start(dbg["dbg_vem"][:, :],
                                                  val_em[0][:].bitcast(F32))

                        agg_ps = psagg.tile([128, FPAD + H], F32, tag="agg")
                        for t in range(ET):
                            for (o, w) in chunk_slices(FPAD + H):
                                nc.tensor.matmul(agg_ps[:, o:o + w], Sb[t][:],
                                                 val_em[t][:, o:o + w],
                                                 start=(t == 0), stop=(t == ET - 1))
                        den = esp.tile([128, H], F32, tag="den")
                        nc.vector.tensor_scalar_add(den[:], agg_ps[:, FPAD:FPAD + H],
                                                    icnt_t[:, b:b + 1])
                        rden = esp.tile([128, H], F32, tag="rden")
                        nc.vector.reciprocal(rden[:], den[:])
                        if i == 0 and b == 0:
                            nc.sync.dma_start(dbg["dbg_rden"][:, :], rden[:])
                        agg_nm = eaggp.tile([128, FPAD], F32, tag="aggnm")
                        nc.vector.tensor_tensor(
                            out=agg_nm[:].rearrange("p (k hh c) -> p k hh c",
                                                    k=KPAD, hh=H, c=4),
                            in0=agg_ps[:, 0:FPAD].rearrange(
                                "p (k hh c) -> p k hh c", k=KPAD, hh=H, c=4),
                            in1=rden[:, None, :, None].to_broadcast([128, KPAD, H, 4]),
                            op=OP.mult)
                        if i == 0 and b == 0:
                            nc.sync.dma_start(dbg["dbg_aggnm"][:, :], agg_nm[:])
                        for m in range(NCH):
                            p = pse.tile([128, 128], F32, tag="pse")
                            nc.tensor.transpose(p[:],
                                                agg_nm[:, m * 128:(m + 1) * 128],
                                                ident[:])
                            aggT = emp.tile([128, 128], F32R, tag="aggT")
                            nc.vector.tensor_copy(aggT[:], p[:])
                            d_ps = pse.tile([128, 128], F32, tag="pse")
                            nc.tensor.matmul(d_ps[:], wr["woutb"][:, m, :], aggT[:],
                                             start=True, stop=True)
                            nc.vector.tensor_tensor(
                                out=x[m][:, b * 128:(b + 1) * 128],
                                in0=x[m][:, b * 128:(b + 1) * 128],
                                in1=d_ps[:], op=OP.add)

            # ================= norm2 =================
            with tc.tile_pool(name=f"wn{i}", bufs=1) as wnpool:
                for nm, shp in [(f"wf1_{i}", [128, KPAD, F]), (f"wf2_{i}", [F, KPAD, 128]),
                                (f"wg_{i}", [C, F])]:
                    t = wnpool.tile(shp, F32R, tag=nm, name=nm)
                    nc.gpsimd.dma_start(t[:], din[nm][:])
                    wr[nm.rsplit("_", 1)[0]] = t
                with tc.tile_pool(name="x2p", bufs=1) as x2p:
                    with tc.tile_pool(name="n2s", bufs=2) as sp, \
                         tc.tile_pool(name="n2p", bufs=2, space="PSUM") as pp, \
                         tc.tile_pool(name="n2st", bufs=1, space="PSUM") as pstat:
                        stat_ps = pstat.tile([NL, NPAD], F32, tag="stat")
                        for m in range(NCH):
                            xsq = sp.tile([128, NPAD], F32R, tag="xsq")
                            nc.scalar.activation(xsq[:], x[m][:], AF.Square)
                            for (o, w) in chunk_slices(NPAD):
                                nc.tensor.matmul(stat_ps[:, o:o + w], statm[:, m, :],
                                                 xsq[:, o:o + w],
                                                 start=(m == 0), stop=(m == NCH - 1))
                        sstat = sp.tile([NL, NPAD], F32, tag="sstat")
                        nc.scalar.activation(sstat[:], stat_ps[:], AF.Sqrt, bias=c1e5[0:NL, :])
                        inv2 = sp.tile([NL, NPAD], F32R, tag="inv2")
                        nc.vector.reciprocal(inv2[:], sstat[:])
                        x2 = [x2p.tile([128, NPAD], F32R, tag=f"x2_{m}", name=f"x2_{m}")
                              for m in range(NCH)]
                        for m in range(NCH):
                            iv_ps = pp.tile([128, NPAD], F32, tag="ivps")
                            for (o, w) in chunk_slices(NPAD):
                                nc.tensor.matmul(iv_ps[:, o:o + w], expm[:, m, :],
                                                 inv2[:, o:o + w],
                                                 start=True, stop=True)
                            nc.vector.tensor_tensor(out=x2[m][:], in0=x[m][:],
                                                    in1=iv_ps[:], op=OP.mult)

                    # ================= FFN =================
                    with tc.tile_pool(name="fs", bufs=3) as fsp, \
                         tc.tile_pool(name="fph", bufs=2, space="PSUM") as fph, \
                         tc.tile_pool(name="fpd", bufs=2, space="PSUM") as fpd:
                        g_ps = fph.tile([F, NPAD], F32, tag="hps")
                        for (o, w) in chunk_slices(NPAD):
                            nc.tensor.matmul(g_ps[:, o:o + w], wr["wg"],
                                             x2[0][0:C, o:o + w],
                                             start=True, stop=True)
                        gateT = x2p.tile([F, NPAD], F32, tag="gateT")
                        nc.scalar.activation(gateT[:], g_ps[:], AF.Silu)
                        for m in range(NCH):
                            d_ps = fpd.tile([128, NPAD], F32, tag="dxps")
                            for j in range(4):
                                k = 4 * m + j
                                h_ps = fph.tile([F, NPAD], F32, tag="hps")
                                for (o, w) in chunk_slices(NPAD):
                                    nc.tensor.matmul(
                                        h_ps[:, o:o + w],
                                        wr["wf1"][:, k, :],
                                        x2[m][:, o:o + w],
                                        start=True, stop=True)
                                hg = fsp.tile([F, NPAD], F32R, tag="hg")
                                nc.vector.tensor_tensor(out=hg[:], in0=gateT[:],
                                                        in1=h_ps[:], op=OP.mult)
                                for (o, w) in chunk_slices(NPAD):
                                    nc.tensor.matmul(
                                        d_ps[:, o:o + w],
                                        wr["wf2"][:, k, :],
                                        hg[:, o:o + w], start=(j == 0), stop=(j == 3))
                            nc.vector.tensor_tensor(out=x[m][:], in0=x[m][:],
                                                    in1=d_ps[:], op=OP.add)

        for m in range(NCH):
            nc.sync.dma_start(xout[m, :, :], x[m][:])
    # note: dbg_xl0 written inside layer loop below is skipped for simplicity

    return nc, list(din.keys())


# ================= entry point =================
_nc_cache = {}


def kernel(**inputs):
    in_maps = make_in_maps(inputs)
    if "nc" not in _nc_cache:
        nc, _ = build_nc()
        split_multiwaits(nc)
        _nc_cache["nc"] = nc
    res = run_bass_kernel_spmd(_nc_cache["nc"], in_maps,
                               core_ids=list(range(NCORE)))
    return unshard(res.results)

